# revision 26
# baseline (speedup 1.0000x reference)
import sys

if "/opt/trn_rl_repo" not in sys.path:
    sys.path.insert(0, "/opt/trn_rl_repo")

from contextlib import ExitStack

import numpy as np

import concourse.mybir as mybir
from concourse import bacc
from concourse.bass_utils import run_bass_kernel_spmd
from concourse.masks import make_identity
from concourse.tile import TileContext

F32 = mybir.dt.float32
F32R = mybir.dt.float32r
BF16 = mybir.dt.bfloat16

B, T, C, H, D = 8, 512, 1024, 16, 64
MAX_POS = 512
TOPK = 32
P = 128
OT = C // P  # 8 channel tiles
TT = T // P  # 4 token tiles
N_CORES = 8

SHIFT = 0.0  # no score shift: removals use -1e30, which works for any sign


def build_program():
    nc = bacc.Bacc(None, target_bir_lowering=False)

    xT_d = nc.declare_dram_parameter("xT", [P, OT, T], F32, isOutput=False)
    # wq/wk are pre-chunked host-side by output tile: [ot_out, P, kt, P]
    wq_d = nc.declare_dram_parameter("wq", [OT, P, OT, P], F32, isOutput=False)
    wk_d = nc.declare_dram_parameter("wk", [OT, P, OT, P], F32, isOutput=False)
    wv_d = nc.declare_dram_parameter("wv", [P, OT, C], F32, isOutput=False)
    wo_d = nc.declare_dram_parameter("wo", [P, OT, C], F32, isOutput=False)
    bqp_d = nc.declare_dram_parameter("bqp", [P, OT], F32, isOutput=False)
    bkp_d = nc.declare_dram_parameter("bkp", [P, OT], F32, isOutput=False)
    bob_d = nc.declare_dram_parameter("bob", [P, C], F32, isOutput=False)
    gates_d = nc.declare_dram_parameter("gates", [P, H], F32, isOutput=False)
    posb_d = nc.declare_dram_parameter("posb", [H, TT, P, T], F32, isOutput=False)
    out_d = nc.declare_dram_parameter("out", [T, C], F32, isOutput=True)

    Exp = mybir.ActivationFunctionType.Exp
    Identity = mybir.ActivationFunctionType.Identity
    Copy = mybir.ActivationFunctionType.Copy
    add = mybir.AluOpType.add
    mult = mybir.AluOpType.mult
    sub_op = mybir.AluOpType.subtract
    is_lt = mybir.AluOpType.is_lt
    is_ge = mybir.AluOpType.is_ge

    with TileContext(nc) as tc, ExitStack() as ctx:
        const = ctx.enter_context(tc.tile_pool(name="const", bufs=1))
        wqkp = ctx.enter_context(tc.tile_pool(name="wqkp", bufs=6))
        wvop = ctx.enter_context(tc.tile_pool(name="wvop", bufs=1))
        xpool = ctx.enter_context(tc.tile_pool(name="xpool", bufs=1))
        proj = ctx.enter_context(tc.tile_pool(name="proj", bufs=1))
        scp = ctx.enter_context(tc.tile_pool(name="scp", bufs=6))
        pbpool = ctx.enter_context(tc.tile_pool(name="pbpool", bufs=4))
        qfpool = ctx.enter_context(tc.tile_pool(name="qfpool", bufs=2))
        empool = ctx.enter_context(tc.tile_pool(name="empool", bufs=3))
        sgpool = ctx.enter_context(tc.tile_pool(name="sgpool", bufs=3))
        epool = ctx.enter_context(tc.tile_pool(name="epool", bufs=5))
        pupool = ctx.enter_context(tc.tile_pool(name="pupool", bufs=12))
        prpool = ctx.enter_context(tc.tile_pool(name="prpool", bufs=8))
        ptpool = ctx.enter_context(tc.tile_pool(name="ptpool", bufs=2))
        small = ctx.enter_context(tc.tile_pool(name="small", bufs=10))
        headp = ctx.enter_context(tc.tile_pool(name="headp", bufs=8))
        outp = ctx.enter_context(tc.tile_pool(name="outp", bufs=2))
        # PSUM (8 banks): psAT shared between projections/transposes/out-proj;
        # psS holds score tiles (pos-bias DMA preload + matmul accumulate);
        # psO per-pair attention-output accumulator.
        psAT = ctx.enter_context(tc.tile_pool(name="psAT", bufs=2, space="PSUM"))
        psS = ctx.enter_context(tc.tile_pool(name="psS", bufs=5, space="PSUM"))
        psO = ctx.enter_context(tc.tile_pool(name="psO", bufs=1, space="PSUM"))

        ident_f = const.tile([P, P], F32)
        make_identity(nc, ident_f)
        ident_b = const.tile([P, P], BF16)
        nc.vector.tensor_copy(ident_b[:], ident_f[:])
        ident_r = const.tile([P, P], F32R)
        nc.vector.tensor_copy(ident_r[:], ident_f[:])
        gates_sb = const.tile([P, H], F32)
        nc.sync.dma_start(gates_sb[:], gates_d[:])
        bqp_sb = const.tile([P, OT], F32)
        nc.sync.dma_start(bqp_sb[:], bqp_d[:])
        bkp_sb = const.tile([P, OT], F32)
        nc.sync.dma_start(bkp_sb[:], bkp_d[:])
        bob_r = const.tile([P, C], F32R)
        nc.gpsimd.dma_start(bob_r[:], bob_d[:].bitcast(F32R))

        xR_sb = xpool.tile([P, OT, T], F32R, tag="x")
        nc.gpsimd.dma_start(xR_sb[:], xT_d[:].bitcast(F32R))

        Qhi = proj.tile([P, OT, T], BF16, tag="qhi")
        Qlo = proj.tile([P, OT, T], BF16, tag="qlo")
        Khi = proj.tile([P, OT, T], BF16, tag="khi")
        Klo = proj.tile([P, OT, T], BF16, tag="klo")
        V_sb = proj.tile([P, TT, C], BF16, tag="v")
        AO_sb = proj.tile([P, OT, T], F32R, tag="ao")

        state = {"wv": None, "wo": None}
        wqk_tiles = {}

        def load_qk_chunk(ot):
            wq_ch = wqkp.tile([P, OT, P], F32R, tag="wqk")
            nc.gpsimd.dma_start(wq_ch[:], wq_d[ot].bitcast(F32R))
            wk_ch = wqkp.tile([P, OT, P], F32R, tag="wqk")
            nc.gpsimd.dma_start(wk_ch[:], wk_d[ot].bitcast(F32R))
            wqk_tiles[ot] = (wq_ch, wk_ch)

        def emit_qk_proj(ot):
            # Q^T / K^T channel tile ot -> bf16 hi/lo splits
            wq_ch, wk_ch = wqk_tiles.pop(ot)
            for w_ch, bias_sb, hi, lo in ((wq_ch, bqp_sb, Qhi, Qlo),
                                          (wk_ch, bkp_sb, Khi, Klo)):
                ps = psAT.tile([P, T], F32, tag="psAT")
                for kt in range(OT):
                    nc.tensor.matmul(
                        ps[:],
                        lhsT=w_ch[:, kt, :],
                        rhs=xR_sb[:, kt, :],
                        start=(kt == 0),
                        stop=(kt == OT - 1),
                    )
                nc.scalar.activation(hi[:, ot, :], ps[:], Identity,
                                     bias=bias_sb[:, ot:ot + 1])
                qf = qfpool.tile([P, T], F32, tag="qf")
                nc.scalar.activation(qf[:], ps[:], Identity,
                                     bias=bias_sb[:, ot:ot + 1])
                nc.gpsimd.tensor_tensor(
                    lo[:, ot, :], qf[:], hi[:, ot, :], op=sub_op)

        def emit_v_group(tt, oh):
            # V[tok, ch] tile (V bias folded into output bias host-side)
            ps = psAT.tile([P, T], F32, tag="psAT")
            for kt in range(OT):
                nc.tensor.matmul(
                    ps[:],
                    lhsT=xR_sb[:, kt, tt * P:(tt + 1) * P],
                    rhs=state["wv"][:, kt, oh * 512:(oh + 1) * 512],
                    start=(kt == 0),
                    stop=(kt == OT - 1),
                )
            nc.scalar.copy(V_sb[:, tt, oh * 512:(oh + 1) * 512], ps[:])

        # ---- tile-granular software pipeline ----
        # Tile t = 8*g + 4*hh + it. Score matmuls for tile t are emitted at
        # step t; its selection/mask lags 2 steps so every engine's in-order
        # queue only sees work whose cross-engine deps are already done.
        tile_ps = {}
        tile_E = {}
        tile_pu = {}
        tile_pb = {}
        pair_sums = {}

        def prefetch_pb(t):
            g, hh, it = t // 8, (t // 4) % 2, t % 4
            h = 2 * g + hh
            pb_sb = pbpool.tile([P, T], F32R, tag="pb")
            nc.sync.dma_start(pb_sb[:], posb_d[h, it].bitcast(F32R))
            tile_pb[t] = pb_sb

        def emit_tile_scores(t):
            g, hh, it = t // 8, (t // 4) % 2, t % 4
            h = 2 * g + hh
            prow = 64 * hh
            if t % 4 == 0:
                pair_sums[(g, hh)] = headp.tile([P, TT], F32, tag="sums",
                                                name="sums_h")
            pb_sb = tile_pb.pop(t)
            ps = psS.tile([P, T], F32, tag="psS")
            nc.tensor.matmul(ps[:], lhsT=ident_r[:], rhs=pb_sb[:],
                             start=True, stop=False)
            terms = ((Qhi, Khi), (Qhi, Klo), (Qlo, Khi))
            for ti, (qq, kk) in enumerate(terms):
                nc.tensor.matmul(
                    ps[:],
                    lhsT=qq[prow:prow + 64, g, it * P:(it + 1) * P],
                    rhs=kk[prow:prow + 64, g, :],
                    start=False,
                    stop=(ti == len(terms) - 1),
                )
            tile_ps[t] = ps

        def emit_E(t):
            E = epool.tile([P, T], F32, tag="E")
            nc.scalar.activation(E[:], tile_ps[t][:], Exp)
            tile_E[t] = E

        def _sel_parts(t):
            g, hh, it = t // 8, (t // 4) % 2, t % 4
            ps = tile_ps.pop(t)
            m_all = small.tile([P, 32], F32, tag="mall", name="m_all")
            scs = [scp.tile([P, T], F32, tag="sc", name="sc") for _ in range(3)]
            return g, hh, it, ps, m_all, scs

        tile_mall = {}

        def emit_select_dve(ts_list):
            # interleave the DVE selection chains of two tiles so consecutive
            # DVE ops never have a same-tile write->read interlock
            parts = [_sel_parts(t) for t in ts_list]
            chains = []
            for t, (g, hh, it, ps, m_all, scs) in zip(ts_list, parts):
                ops = []
                srcs = [ps] + scs
                for r in range(4):
                    ops.append(("max", m_all, r, srcs[r]))
                    if r < 3:
                        ops.append(("mr", m_all, r, srcs[r], srcs[r + 1]))
                chains.append(ops)
                tile_mall[t] = m_all
            for i in range(7):
                for ops in chains:
                    op = ops[i]
                    if op[0] == "max":
                        _, m_all, r, s_in = op
                        nc.vector.max(out=m_all[:, 8 * r:8 * r + 8], in_=s_in[:])
                    else:
                        _, m_all, r, s_in, s_out = op
                        nc.vector.match_replace(
                            out=s_out[:], in_to_replace=m_all[:, 8 * r:8 * r + 8],
                            in_values=s_in[:], imm_value=-1e30)

        def emit_select_post(ts_list):
            for t in ts_list:
                g, hh, it = t // 8, (t // 4) % 2, t % 4
                m_all = tile_mall.pop(t)
                E = tile_E.pop(t)
                sums_h = pair_sums[(g, hh)]
                scrapM = small.tile([P, 32], F32, tag="scrapM")
                nc.scalar.activation(
                    scrapM[:], m_all[:], Exp,
                    accum_out=sums_h[:, it:it + 1])
                sgn = sgpool.tile([P, T], BF16, tag="sgn")
                nc.scalar.activation(sgn[:], E[:],
                                     mybir.ActivationFunctionType.Sign,
                                     bias=scrapM[:, 31:32],
                                     scale=-(1.0 + 5e-6))
                Em = empool.tile([P, T], F32, tag="Em")
                nc.gpsimd.tensor_tensor(Em[:], E[:], sgn[:], op=mult)
                p_u = pupool.tile([P, T], BF16, tag="P")
                nc.gpsimd.tensor_tensor(p_u[:], E[:], Em[:], op=sub_op)
                tile_pu[t] = p_u

        tail_state = {}

        def tail_scale(g, hh):
            # reciprocal + gate scale for one head (tiny DVE ops)
            h = 2 * g + hh
            sums_h = pair_sums.pop((g, hh))
            inv = headp.tile([P, TT], F32, tag="inv")
            nc.vector.reciprocal(inv[:], sums_h[:])
            scl = headp.tile([P, TT], F32, tag="scl")
            nc.vector.tensor_scalar(scl[:], inv[:],
                                    gates_sb[:, h:h + 1], 0.5,
                                    op0=mult, op1=mult)
            tail_state[(g, hh, "scl")] = scl

        def tail_pr(g, hh):
            scl = tail_state.pop((g, hh, "scl"))
            p_r = []
            for it in range(TT):
                pr = prpool.tile([P, T], BF16, tag="Pr")
                nc.scalar.activation(pr[:], tile_pu.pop(8 * g + 4 * hh + it)[:],
                                     Copy, scale=scl[:, it:it + 1])
                p_r.append(pr)
            tail_state[(g, hh, "pr")] = p_r

        def tail_transpose(g, hh):
            p_r = tail_state.pop((g, hh, "pr"))
            pts = []
            for jt in range(TT):
                pt_ps = psAT.tile([P, T], BF16, tag="psAT", name="pt_ps")
                for it in range(TT):
                    nc.tensor.transpose(
                        pt_ps[:, it * P:(it + 1) * P],
                        p_r[it][:, jt * P:(jt + 1) * P],
                        ident_b[:],
                    )
                PT_sb = ptpool.tile([P, T], BF16, tag="PT")
                nc.scalar.copy(PT_sb[:], pt_ps[:])
                pts.append(PT_sb)
            tail_state[(g, hh, "pt")] = pts

        def tail_ao(g, hh):
            h = 2 * g + hh
            pts = tail_state.pop((g, hh, "pt"))
            if hh == 0:
                ao_ps = psO.tile([P, T], F32, tag="psO")
                tail_state[(g, "ao")] = ao_ps
            else:
                ao_ps = tail_state[(g, "ao")]
            for jt in range(TT):
                nc.tensor.matmul(
                    ao_ps[64 * hh:64 * hh + 64, :],
                    lhsT=V_sb[:, jt, h * 64:(h + 1) * 64],
                    rhs=pts[jt][:],
                    start=(jt == 0),
                    stop=(jt == TT - 1),
                )
            if hh == 1:
                ao_ps = tail_state.pop((g, "ao"))
                nc.scalar.copy(AO_sb[:, g, :], ao_ps[:])

        # schedules (step -> work), keeping PE fed without starving the
        # selection pipeline
        qk_sched = {}
        for ot in range(2, OT):
            s0 = 8 * (ot - 1) + 1
            qk_sched.setdefault(s0, []).append((ot, 0, 0))
            qk_sched.setdefault(s0 + 2, []).append((ot, 0, 1))
            qk_sched.setdefault(s0 + 4, []).append((ot, 1, 0))
            qk_sched.setdefault(s0 + 6, []).append((ot, 1, 1))
        chunk_sched = {8 * (ot - 2): ot for ot in range(2, OT)}
        v_sched = {5: (0, 0), 7: (1, 0), 9: (2, 0), 10: (3, 0),
                   18: (0, 1), 20: (1, 1), 22: (2, 1), 24: (3, 1)}

        load_qk_chunk(0)
        load_qk_chunk(1)
        state["wv"] = wvop.tile([P, OT, C], F32R, tag="wvo", name="wv_sb")
        nc.gpsimd.dma_start(state["wv"][:], wv_d[:].bitcast(F32R))
        emit_qk_proj(0)
        emit_qk_proj(1)

        qk_partial = {}

        def emit_qk_half(ot, which, half):
            # half of one (q, k) projection group for channel tile ot;
            # splitting keeps PE bursts short so score tiles aren't delayed
            wq_ch, wk_ch = wqk_tiles[ot]
            w_ch, bias_sb, hi, lo = ((wq_ch, bqp_sb, Qhi, Qlo),
                                     (wk_ch, bkp_sb, Khi, Klo))[which]
            if half == 0:
                ps = psAT.tile([P, T], F32, tag="psAT", name="qk_ps")
                qk_partial[(ot, which)] = ps
            else:
                ps = qk_partial.pop((ot, which))
            for kt in range(4 * half, 4 * half + 4):
                nc.tensor.matmul(
                    ps[:],
                    lhsT=w_ch[:, kt, :],
                    rhs=xR_sb[:, kt, :],
                    start=(kt == 0),
                    stop=(kt == OT - 1),
                )
            if half == 1:
                nc.scalar.activation(hi[:, ot, :], ps[:], Identity,
                                     bias=bias_sb[:, ot:ot + 1])
                qf = qfpool.tile([P, T], F32, tag="qf")
                nc.scalar.activation(qf[:], ps[:], Identity,
                                     bias=bias_sb[:, ot:ot + 1])
                nc.gpsimd.tensor_tensor(
                    lo[:, ot, :], qf[:], hi[:, ot, :], op=sub_op)
                if which == 1:
                    wqk_tiles.pop(ot)

        # tail stage schedule keyed by step s: stage k of (pair g, head hh)
        # runs at s = 8g + 4*hh + 8 + k   (scale, p_r, transpose, AO)
        tail_stages = {}
        for g in range(OT):
            for hh in range(2):
                base = 8 * g + 4 * hh + 9
                for k, fn in enumerate((tail_scale, tail_pr,
                                        tail_transpose, tail_ao)):
                    tail_stages.setdefault(base + k, []).append((fn, g, hh))

        prefetch_pb(0)
        prefetch_pb(1)
        for s in range(73):
            if s < 64:
                emit_tile_scores(s)
            if 0 <= s - 1 < 64:
                emit_E(s - 1)
            t = s - 3
            if 0 <= t < 64 and t % 2 == 1:
                emit_select_dve([t - 1, t])
            u = s - 4
            if 0 <= u < 64 and u % 2 == 1:
                emit_select_post([u - 1, u])
            for fn, g, hh in tail_stages.get(s, ()):
                fn(g, hh)
            if s + 2 < 64:
                prefetch_pb(s + 2)
            if s in chunk_sched:
                load_qk_chunk(chunk_sched[s])
            for ot, which, half in qk_sched.get(s, ()):
                emit_qk_half(ot, which, half)
            if s in v_sched:
                emit_v_group(*v_sched[s])
            if s == 30:
                state["wo"] = wvop.tile([P, OT, C], F32R, tag="wvo",
                                        name="wo_sb")
                nc.gpsimd.dma_start(state["wo"][:], wo_d[:].bitcast(F32R))

        # ---- output projection (f32r) ----
        for tt in range(TT):
            for oh in range(2):
                ps = psAT.tile([P, T], F32, tag="psAT")
                nc.tensor.matmul(ps[:], lhsT=ident_r[:],
                                 rhs=bob_r[:, oh * 512:(oh + 1) * 512],
                                 start=True, stop=False)
                for ct in range(OT):
                    nc.tensor.matmul(
                        ps[:],
                        lhsT=AO_sb[:, ct, tt * P:(tt + 1) * P],
                        rhs=state["wo"][:, ct, oh * 512:(oh + 1) * 512],
                        start=False,
                        stop=(ct == OT - 1),
                    )
                o_sb = outp.tile([P, T], F32, tag="o")
                nc.scalar.copy(o_sb[:], ps[:])
                nc.sync.dma_start(out_d[tt * P:(tt + 1) * P,
                                        oh * 512:(oh + 1) * 512], o_sb[:])

    nc.compile()
    if not nc.is_finalized():
        nc.finalize()
    return nc


def prep_inputs(x, Wq, bq, Wk, bk, Wv, bv, Wo, bo, head_gates, rel_bias):
    """Host-side reshapes/transposes into the layouts the device program wants."""
    x = np.asarray(x, np.float32)
    scale = np.float32(1.0 / np.sqrt(D))

    def to_kpart(w):
        # [C_in, C_out] -> [P, OT, C_out] with c_in = kt*P + p
        return np.ascontiguousarray(
            np.asarray(w, np.float32).reshape(OT, P, C).transpose(1, 0, 2))

    def to_kpart_chunked(w):
        # [C_in, C_out] -> [OT_out, P, OT_kt, P]
        return np.ascontiguousarray(
            np.asarray(w, np.float32).reshape(OT, P, OT, P)
            .transpose(2, 1, 0, 3))

    wq_r = to_kpart_chunked(np.asarray(Wq, np.float32).T * scale)
    wk_r = to_kpart_chunked(np.asarray(Wk, np.float32).T)
    wv_r = to_kpart(np.asarray(Wv, np.float32).T)
    wo_r = to_kpart(np.asarray(Wo, np.float32).T)

    bqp = np.ascontiguousarray((np.asarray(bq, np.float32) * scale).reshape(OT, P).T)
    bkp = np.ascontiguousarray(np.asarray(bk, np.float32).reshape(OT, P).T)
    # V bias folded into the output bias: the normalized gated weights of each
    # head sum to exactly gate_h, so attn_out carries a constant gate_h * bv_h
    # per head, which maps through Wo^T into a constant output bias.
    g64 = np.repeat(np.asarray(head_gates, np.float64), D)
    bo_eff = (np.asarray(bo, np.float64)
              + (g64 * np.asarray(bv, np.float64)) @ np.asarray(Wo, np.float64).T)
    bob = np.ascontiguousarray(
        np.tile(bo_eff.astype(np.float32)[None, :], (P, 1)))
    gates = np.ascontiguousarray(
        np.tile(np.asarray(head_gates, np.float32)[None, :], (P, 1)))

    idx = np.arange(T)
    rel = idx[None, :] - idx[:, None] + (MAX_POS - 1)          # [T, T]
    pb = np.asarray(rel_bias, np.float32)[rel] + np.float32(SHIFT)  # [T, T, H]
    posb = np.ascontiguousarray(
        pb.transpose(2, 0, 1).reshape(H, TT, P, T))            # [H, TT, P, T]

    shared = dict(wq=wq_r, wk=wk_r, wv=wv_r, wo=wo_r, bqp=bqp, bkp=bkp,
                  bob=bob, gates=gates, posb=posb)

    in_maps = []
    for b in range(B):
        xT = np.ascontiguousarray(
            x[b].T.reshape(OT, P, T).transpose(1, 0, 2))       # [P, OT, T]
        in_maps.append(dict(xT=xT, **shared))
    return in_maps


_NC_CACHE = {}


def get_program():
    if "nc" not in _NC_CACHE:
        _NC_CACHE["nc"] = build_program()
    return _NC_CACHE["nc"]


def kernel(x, Wq, bq, Wk, bk, Wv, bv, Wo, bo, head_gates, rel_bias):
    nc = get_program()
    in_maps = prep_inputs(x, Wq, bq, Wk, bk, Wv, bv, Wo, bo, head_gates, rel_bias)
    res = run_bass_kernel_spmd(nc, in_maps, list(range(N_CORES)))
    return np.stack([res.results[b]["out"] for b in range(B)], axis=0)


# revision 27
# speedup vs baseline: 1.2106x; 1.2106x over previous
import sys

if "/opt/trn_rl_repo" not in sys.path:
    sys.path.insert(0, "/opt/trn_rl_repo")

from contextlib import ExitStack

import numpy as np

import concourse.mybir as mybir
from concourse import bacc
from concourse.bass_utils import run_bass_kernel_spmd
from concourse.masks import make_identity
from concourse.tile import TileContext

F32 = mybir.dt.float32
F32R = mybir.dt.float32r
BF16 = mybir.dt.bfloat16

B, T, C, H, D = 8, 512, 1024, 16, 64
MAX_POS = 512
TOPK = 32
P = 128
OT = C // P  # 8 channel tiles
TT = T // P  # 4 token tiles
N_CORES = 8

SHIFT = 0.0  # no score shift: removals use -1e30, which works for any sign


def build_program():
    nc = bacc.Bacc(None, target_bir_lowering=False)

    xT_d = nc.declare_dram_parameter("xT", [P, OT, T], F32, isOutput=False)
    # wq/wk are pre-chunked host-side by output tile: [ot_out, P, kt, P]
    wq_d = nc.declare_dram_parameter("wq", [OT, P, OT, P], F32, isOutput=False)
    wk_d = nc.declare_dram_parameter("wk", [OT, P, OT, P], F32, isOutput=False)
    wv_d = nc.declare_dram_parameter("wv", [P, OT, C], F32, isOutput=False)
    wo_d = nc.declare_dram_parameter("wo", [P, OT, C], F32, isOutput=False)
    bqp_d = nc.declare_dram_parameter("bqp", [P, OT], F32, isOutput=False)
    bkp_d = nc.declare_dram_parameter("bkp", [P, OT], F32, isOutput=False)
    bob_d = nc.declare_dram_parameter("bob", [P, C], F32, isOutput=False)
    gates_d = nc.declare_dram_parameter("gates", [P, H], F32, isOutput=False)
    posb_d = nc.declare_dram_parameter("posb", [H, TT, P, T], F32, isOutput=False)
    out_d = nc.declare_dram_parameter("out", [T, C], F32, isOutput=True)

    Exp = mybir.ActivationFunctionType.Exp
    Identity = mybir.ActivationFunctionType.Identity
    Copy = mybir.ActivationFunctionType.Copy
    add = mybir.AluOpType.add
    mult = mybir.AluOpType.mult
    sub_op = mybir.AluOpType.subtract
    is_lt = mybir.AluOpType.is_lt
    is_ge = mybir.AluOpType.is_ge

    with TileContext(nc) as tc, ExitStack() as ctx:
        const = ctx.enter_context(tc.tile_pool(name="const", bufs=1))
        wqkp = ctx.enter_context(tc.tile_pool(name="wqkp", bufs=6))
        wvop = ctx.enter_context(tc.tile_pool(name="wvop", bufs=1))
        xpool = ctx.enter_context(tc.tile_pool(name="xpool", bufs=1))
        proj = ctx.enter_context(tc.tile_pool(name="proj", bufs=1))
        scp = ctx.enter_context(tc.tile_pool(name="scp", bufs=6))
        pbpool = ctx.enter_context(tc.tile_pool(name="pbpool", bufs=4))
        qfpool = ctx.enter_context(tc.tile_pool(name="qfpool", bufs=2))
        empool = ctx.enter_context(tc.tile_pool(name="empool", bufs=3))
        sgpool = ctx.enter_context(tc.tile_pool(name="sgpool", bufs=3))
        epool = ctx.enter_context(tc.tile_pool(name="epool", bufs=5))
        pupool = ctx.enter_context(tc.tile_pool(name="pupool", bufs=12))
        prpool = ctx.enter_context(tc.tile_pool(name="prpool", bufs=8))
        ptpool = ctx.enter_context(tc.tile_pool(name="ptpool", bufs=2))
        small = ctx.enter_context(tc.tile_pool(name="small", bufs=10))
        headp = ctx.enter_context(tc.tile_pool(name="headp", bufs=8))
        outp = ctx.enter_context(tc.tile_pool(name="outp", bufs=2))
        # PSUM (8 banks): psAT shared between projections/transposes/out-proj;
        # psS holds score tiles (pos-bias DMA preload + matmul accumulate);
        # psO per-pair attention-output accumulator.
        psAT = ctx.enter_context(tc.tile_pool(name="psAT", bufs=2, space="PSUM"))
        psS = ctx.enter_context(tc.tile_pool(name="psS", bufs=5, space="PSUM"))
        psO = ctx.enter_context(tc.tile_pool(name="psO", bufs=1, space="PSUM"))

        ident_f = const.tile([P, P], F32)
        make_identity(nc, ident_f)
        ident_b = const.tile([P, P], BF16)
        nc.vector.tensor_copy(ident_b[:], ident_f[:])
        ident_r = const.tile([P, P], F32R)
        nc.vector.tensor_copy(ident_r[:], ident_f[:])
        gates_sb = const.tile([P, H], F32)
        nc.sync.dma_start(gates_sb[:], gates_d[:])
        bqp_sb = const.tile([P, OT], F32)
        nc.sync.dma_start(bqp_sb[:], bqp_d[:])
        bkp_sb = const.tile([P, OT], F32)
        nc.sync.dma_start(bkp_sb[:], bkp_d[:])
        bob_r = const.tile([P, C], F32R)
        nc.gpsimd.dma_start(bob_r[:], bob_d[:].bitcast(F32R))

        xR_sb = xpool.tile([P, OT, T], F32R, tag="x")
        nc.gpsimd.dma_start(xR_sb[:], xT_d[:].bitcast(F32R))

        Qhi = proj.tile([P, OT, T], BF16, tag="qhi")
        Qlo = proj.tile([P, OT, T], BF16, tag="qlo")
        Khi = proj.tile([P, OT, T], BF16, tag="khi")
        Klo = proj.tile([P, OT, T], BF16, tag="klo")
        V_sb = proj.tile([P, TT, C], BF16, tag="v")
        AO_sb = proj.tile([P, OT, T], F32R, tag="ao")

        state = {"wv": None, "wo": None}
        wqk_tiles = {}

        def load_qk_chunk(ot):
            wq_ch = wqkp.tile([P, OT, P], F32R, tag="wqk")
            nc.gpsimd.dma_start(wq_ch[:], wq_d[ot].bitcast(F32R))
            wk_ch = wqkp.tile([P, OT, P], F32R, tag="wqk")
            nc.gpsimd.dma_start(wk_ch[:], wk_d[ot].bitcast(F32R))
            wqk_tiles[ot] = (wq_ch, wk_ch)

        def emit_qk_proj(ot):
            # Q^T / K^T channel tile ot -> bf16 hi/lo splits
            wq_ch, wk_ch = wqk_tiles.pop(ot)
            for w_ch, bias_sb, hi, lo in ((wq_ch, bqp_sb, Qhi, Qlo),
                                          (wk_ch, bkp_sb, Khi, Klo)):
                ps = psAT.tile([P, T], F32, tag="psAT")
                for kt in range(OT):
                    nc.tensor.matmul(
                        ps[:],
                        lhsT=w_ch[:, kt, :],
                        rhs=xR_sb[:, kt, :],
                        start=(kt == 0),
                        stop=(kt == OT - 1),
                    )
                nc.scalar.activation(hi[:, ot, :], ps[:], Identity,
                                     bias=bias_sb[:, ot:ot + 1])
                qf = qfpool.tile([P, T], F32, tag="qf")
                nc.scalar.activation(qf[:], ps[:], Identity,
                                     bias=bias_sb[:, ot:ot + 1])
                nc.gpsimd.tensor_tensor(
                    lo[:, ot, :], qf[:], hi[:, ot, :], op=sub_op)

        def emit_v_group(tt, oh):
            # V[tok, ch] tile (V bias folded into output bias host-side)
            ps = psAT.tile([P, T], F32, tag="psAT")
            for kt in range(OT):
                nc.tensor.matmul(
                    ps[:],
                    lhsT=xR_sb[:, kt, tt * P:(tt + 1) * P],
                    rhs=state["wv"][:, kt, oh * 512:(oh + 1) * 512],
                    start=(kt == 0),
                    stop=(kt == OT - 1),
                )
            nc.scalar.copy(V_sb[:, tt, oh * 512:(oh + 1) * 512], ps[:])

        # ---- tile-granular software pipeline ----
        # Tile t = 8*g + 4*hh + it. Score matmuls for tile t are emitted at
        # step t; its selection/mask lags 2 steps so every engine's in-order
        # queue only sees work whose cross-engine deps are already done.
        tile_ps = {}
        tile_E = {}
        tile_pu = {}
        tile_pb = {}
        pair_sums = {}

        def prefetch_pb(t):
            g, hh, it = t // 8, (t // 4) % 2, t % 4
            h = 2 * g + hh
            pb_sb = pbpool.tile([P, T], F32R, tag="pb")
            nc.sync.dma_start(pb_sb[:], posb_d[h, it].bitcast(F32R))
            tile_pb[t] = pb_sb

        def emit_tile_scores(t):
            g, hh, it = t // 8, (t // 4) % 2, t % 4
            h = 2 * g + hh
            prow = 64 * hh
            if t % 4 == 0:
                pair_sums[(g, hh)] = headp.tile([P, TT], F32, tag="sums",
                                                name="sums_h")
            pb_sb = tile_pb.pop(t)
            ps = psS.tile([P, T], F32, tag="psS")
            nc.tensor.matmul(ps[:], lhsT=ident_r[:], rhs=pb_sb[:],
                             start=True, stop=False)
            terms = ((Qhi, Khi), (Qhi, Klo), (Qlo, Khi))
            for ti, (qq, kk) in enumerate(terms):
                nc.tensor.matmul(
                    ps[:],
                    lhsT=qq[prow:prow + 64, g, it * P:(it + 1) * P],
                    rhs=kk[prow:prow + 64, g, :],
                    start=False,
                    stop=(ti == len(terms) - 1),
                )
            tile_ps[t] = ps

        def emit_E(t):
            E = epool.tile([P, T], F32, tag="E")
            nc.scalar.activation(E[:], tile_ps[t][:], Exp)
            tile_E[t] = E

        def _sel_parts(t):
            g, hh, it = t // 8, (t // 4) % 2, t % 4
            ps = tile_ps.pop(t)
            m_all = small.tile([P, 32], F32, tag="mall", name="m_all")
            scs = [scp.tile([P, T], F32, tag="sc", name="sc") for _ in range(3)]
            return g, hh, it, ps, m_all, scs

        tile_mall = {}

        def emit_select_dve(ts_list):
            # interleave the DVE selection chains of two tiles so consecutive
            # DVE ops never have a same-tile write->read interlock
            parts = [_sel_parts(t) for t in ts_list]
            chains = []
            for t, (g, hh, it, ps, m_all, scs) in zip(ts_list, parts):
                ops = []
                srcs = [ps] + scs
                for r in range(4):
                    ops.append(("max", m_all, r, srcs[r]))
                    if r < 3:
                        ops.append(("mr", m_all, r, srcs[r], srcs[r + 1]))
                chains.append(ops)
                tile_mall[t] = m_all
            for i in range(7):
                for ops in chains:
                    op = ops[i]
                    if op[0] == "max":
                        _, m_all, r, s_in = op
                        nc.vector.max(out=m_all[:, 8 * r:8 * r + 8], in_=s_in[:])
                    else:
                        _, m_all, r, s_in, s_out = op
                        nc.vector.match_replace(
                            out=s_out[:], in_to_replace=m_all[:, 8 * r:8 * r + 8],
                            in_values=s_in[:], imm_value=-1e30)

        def emit_select_post(ts_list):
            for t in ts_list:
                g, hh, it = t // 8, (t // 4) % 2, t % 4
                m_all = tile_mall.pop(t)
                E = tile_E.pop(t)
                sums_h = pair_sums[(g, hh)]
                scrapM = small.tile([P, 32], F32, tag="scrapM")
                nc.scalar.activation(
                    scrapM[:], m_all[:], Exp,
                    accum_out=sums_h[:, it:it + 1])
                sgn = sgpool.tile([P, T], BF16, tag="sgn")
                nc.scalar.activation(sgn[:], E[:],
                                     mybir.ActivationFunctionType.Sign,
                                     bias=scrapM[:, 31:32],
                                     scale=-(1.0 + 5e-6))
                Em = empool.tile([P, T], F32, tag="Em")
                nc.gpsimd.tensor_tensor(Em[:], E[:], sgn[:], op=mult)
                p_u = pupool.tile([P, T], BF16, tag="P")
                nc.gpsimd.tensor_tensor(p_u[:], E[:], Em[:], op=sub_op)
                tile_pu[t] = p_u

        tail_state = {}

        def tail_scale(g, hh):
            # reciprocal + gate scale for one head (tiny DVE ops)
            h = 2 * g + hh
            sums_h = pair_sums.pop((g, hh))
            inv = headp.tile([P, TT], F32, tag="inv")
            nc.vector.reciprocal(inv[:], sums_h[:])
            scl = headp.tile([P, TT], F32, tag="scl")
            nc.vector.tensor_scalar(scl[:], inv[:],
                                    gates_sb[:, h:h + 1], 0.5,
                                    op0=mult, op1=mult)
            tail_state[(g, hh, "scl")] = scl

        def tail_pr(g, hh):
            scl = tail_state.pop((g, hh, "scl"))
            p_r = []
            for it in range(TT):
                pr = prpool.tile([P, T], BF16, tag="Pr")
                nc.scalar.activation(pr[:], tile_pu.pop(8 * g + 4 * hh + it)[:],
                                     Copy, scale=scl[:, it:it + 1])
                p_r.append(pr)
            tail_state[(g, hh, "pr")] = p_r

        def tail_transpose(g, hh):
            p_r = tail_state.pop((g, hh, "pr"))
            pts = []
            for jt in range(TT):
                pt_ps = psAT.tile([P, T], BF16, tag="psAT", name="pt_ps")
                for it in range(TT):
                    nc.tensor.transpose(
                        pt_ps[:, it * P:(it + 1) * P],
                        p_r[it][:, jt * P:(jt + 1) * P],
                        ident_b[:],
                    )
                PT_sb = ptpool.tile([P, T], BF16, tag="PT")
                nc.scalar.copy(PT_sb[:], pt_ps[:])
                pts.append(PT_sb)
            tail_state[(g, hh, "pt")] = pts

        def tail_ao(g, hh):
            h = 2 * g + hh
            pts = tail_state.pop((g, hh, "pt"))
            if hh == 0:
                ao_ps = psO.tile([P, T], F32, tag="psO")
                tail_state[(g, "ao")] = ao_ps
            else:
                ao_ps = tail_state[(g, "ao")]
            for jt in range(TT):
                nc.tensor.matmul(
                    ao_ps[64 * hh:64 * hh + 64, :],
                    lhsT=V_sb[:, jt, h * 64:(h + 1) * 64],
                    rhs=pts[jt][:],
                    start=(jt == 0),
                    stop=(jt == TT - 1),
                )
            if hh == 1:
                ao_ps = tail_state.pop((g, "ao"))
                nc.scalar.copy(AO_sb[:, g, :], ao_ps[:])

        # schedules (step -> work), keeping PE fed without starving the
        # selection pipeline
        qk_sched = {}
        for ot in range(2, OT):
            s0 = 8 * (ot - 1) + 1
            qk_sched.setdefault(s0, []).append((ot, 0, 0))
            qk_sched.setdefault(s0 + 2, []).append((ot, 0, 1))
            qk_sched.setdefault(s0 + 4, []).append((ot, 1, 0))
            qk_sched.setdefault(s0 + 6, []).append((ot, 1, 1))
        chunk_sched = {8 * (ot - 2): ot for ot in range(2, OT)}
        v_sched = {2: (0, 0), 4: (1, 0), 6: (2, 0), 8: (3, 0),
                   18: (0, 1), 20: (1, 1), 22: (2, 1), 24: (3, 1)}

        load_qk_chunk(0)
        load_qk_chunk(1)
        state["wv"] = wvop.tile([P, OT, C], F32R, tag="wvo", name="wv_sb")
        nc.gpsimd.dma_start(state["wv"][:], wv_d[:].bitcast(F32R))
        emit_qk_proj(0)
        emit_qk_proj(1)

        qk_partial = {}

        def emit_qk_half(ot, which, half):
            # half of one (q, k) projection group for channel tile ot;
            # splitting keeps PE bursts short so score tiles aren't delayed
            wq_ch, wk_ch = wqk_tiles[ot]
            w_ch, bias_sb, hi, lo = ((wq_ch, bqp_sb, Qhi, Qlo),
                                     (wk_ch, bkp_sb, Khi, Klo))[which]
            if half == 0:
                ps = psAT.tile([P, T], F32, tag="psAT", name="qk_ps")
                qk_partial[(ot, which)] = ps
            else:
                ps = qk_partial.pop((ot, which))
            for kt in range(4 * half, 4 * half + 4):
                nc.tensor.matmul(
                    ps[:],
                    lhsT=w_ch[:, kt, :],
                    rhs=xR_sb[:, kt, :],
                    start=(kt == 0),
                    stop=(kt == OT - 1),
                )
            if half == 1:
                nc.scalar.activation(hi[:, ot, :], ps[:], Identity,
                                     bias=bias_sb[:, ot:ot + 1])
                qf = qfpool.tile([P, T], F32, tag="qf")
                nc.scalar.activation(qf[:], ps[:], Identity,
                                     bias=bias_sb[:, ot:ot + 1])
                nc.gpsimd.tensor_tensor(
                    lo[:, ot, :], qf[:], hi[:, ot, :], op=sub_op)
                if which == 1:
                    wqk_tiles.pop(ot)

        # tail stage schedule keyed by step s: stage k of (pair g, head hh)
        # runs at s = 8g + 4*hh + 8 + k   (scale, p_r, transpose, AO)
        tail_stages = {}
        for g in range(OT):
            for hh in range(2):
                base = 8 * g + 4 * hh + 8
                for k, fn in enumerate((tail_scale, tail_pr,
                                        tail_transpose, tail_ao)):
                    tail_stages.setdefault(base + k, []).append((fn, g, hh))

        prefetch_pb(0)
        prefetch_pb(1)
        for s in range(72):
            if s < 64:
                emit_tile_scores(s)
            if 0 <= s - 1 < 64:
                emit_E(s - 1)
            t = s - 3
            if 0 <= t < 64 and t % 2 == 1:
                emit_select_dve([t - 1, t])
            u = s - 4
            if 0 <= u < 64 and u % 2 == 1:
                emit_select_post([u - 1, u])
            for fn, g, hh in tail_stages.get(s, ()):
                fn(g, hh)
            if s + 2 < 64:
                prefetch_pb(s + 2)
            if s in chunk_sched:
                load_qk_chunk(chunk_sched[s])
            for ot, which, half in qk_sched.get(s, ()):
                emit_qk_half(ot, which, half)
            if s in v_sched:
                emit_v_group(*v_sched[s])
            if s == 30:
                state["wo"] = wvop.tile([P, OT, C], F32R, tag="wvo",
                                        name="wo_sb")
                nc.gpsimd.dma_start(state["wo"][:], wo_d[:].bitcast(F32R))

        # ---- output projection (f32r) ----
        for tt in range(TT):
            for oh in range(2):
                ps = psAT.tile([P, T], F32, tag="psAT")
                nc.tensor.matmul(ps[:], lhsT=ident_r[:],
                                 rhs=bob_r[:, oh * 512:(oh + 1) * 512],
                                 start=True, stop=False)
                for ct in range(OT):
                    nc.tensor.matmul(
                        ps[:],
                        lhsT=AO_sb[:, ct, tt * P:(tt + 1) * P],
                        rhs=state["wo"][:, ct, oh * 512:(oh + 1) * 512],
                        start=False,
                        stop=(ct == OT - 1),
                    )
                o_sb = outp.tile([P, T], F32, tag="o")
                nc.scalar.copy(o_sb[:], ps[:])
                nc.sync.dma_start(out_d[tt * P:(tt + 1) * P,
                                        oh * 512:(oh + 1) * 512], o_sb[:])

    nc.compile()
    if not nc.is_finalized():
        nc.finalize()
    return nc


def prep_inputs(x, Wq, bq, Wk, bk, Wv, bv, Wo, bo, head_gates, rel_bias):
    """Host-side reshapes/transposes into the layouts the device program wants."""
    x = np.asarray(x, np.float32)
    scale = np.float32(1.0 / np.sqrt(D))

    def to_kpart(w):
        # [C_in, C_out] -> [P, OT, C_out] with c_in = kt*P + p
        return np.ascontiguousarray(
            np.asarray(w, np.float32).reshape(OT, P, C).transpose(1, 0, 2))

    def to_kpart_chunked(w):
        # [C_in, C_out] -> [OT_out, P, OT_kt, P]
        return np.ascontiguousarray(
            np.asarray(w, np.float32).reshape(OT, P, OT, P)
            .transpose(2, 1, 0, 3))

    wq_r = to_kpart_chunked(np.asarray(Wq, np.float32).T * scale)
    wk_r = to_kpart_chunked(np.asarray(Wk, np.float32).T)
    wv_r = to_kpart(np.asarray(Wv, np.float32).T)
    wo_r = to_kpart(np.asarray(Wo, np.float32).T)

    bqp = np.ascontiguousarray((np.asarray(bq, np.float32) * scale).reshape(OT, P).T)
    bkp = np.ascontiguousarray(np.asarray(bk, np.float32).reshape(OT, P).T)
    # V bias folded into the output bias: the normalized gated weights of each
    # head sum to exactly gate_h, so attn_out carries a constant gate_h * bv_h
    # per head, which maps through Wo^T into a constant output bias.
    g64 = np.repeat(np.asarray(head_gates, np.float64), D)
    bo_eff = (np.asarray(bo, np.float64)
              + (g64 * np.asarray(bv, np.float64)) @ np.asarray(Wo, np.float64).T)
    bob = np.ascontiguousarray(
        np.tile(bo_eff.astype(np.float32)[None, :], (P, 1)))
    gates = np.ascontiguousarray(
        np.tile(np.asarray(head_gates, np.float32)[None, :], (P, 1)))

    idx = np.arange(T)
    rel = idx[None, :] - idx[:, None] + (MAX_POS - 1)          # [T, T]
    pb = np.asarray(rel_bias, np.float32)[rel] + np.float32(SHIFT)  # [T, T, H]
    posb = np.ascontiguousarray(
        pb.transpose(2, 0, 1).reshape(H, TT, P, T))            # [H, TT, P, T]

    shared = dict(wq=wq_r, wk=wk_r, wv=wv_r, wo=wo_r, bqp=bqp, bkp=bkp,
                  bob=bob, gates=gates, posb=posb)

    in_maps = []
    for b in range(B):
        xT = np.ascontiguousarray(
            x[b].T.reshape(OT, P, T).transpose(1, 0, 2))       # [P, OT, T]
        in_maps.append(dict(xT=xT, **shared))
    return in_maps


_NC_CACHE = {}


def get_program():
    if "nc" not in _NC_CACHE:
        _NC_CACHE["nc"] = build_program()
    return _NC_CACHE["nc"]


def kernel(x, Wq, bq, Wk, bk, Wv, bv, Wo, bo, head_gates, rel_bias):
    nc = get_program()
    in_maps = prep_inputs(x, Wq, bq, Wk, bk, Wv, bv, Wo, bo, head_gates, rel_bias)
    res = run_bass_kernel_spmd(nc, in_maps, list(range(N_CORES)))
    return np.stack([res.results[b]["out"] for b in range(B)], axis=0)
